# revision 30
# baseline (speedup 1.0000x reference)
"""Trainium2 Bass kernel for CrossModalAttention (fp8 v2).

Reference computation (per modality pair):
    ctx_a = softmax((Wq xa)^T (Wk xb) * rdim^-0.5) applied to (Wv xb)
    enh_a = BatchNorm(xa + ctx_a)   # training-mode BN, stats over (B, H, W)

Sharding: 8 cores <- 8 independent (batch, modality) attention problems
(B=4 x 2 modalities).  BatchNorm stats sync via AllReduce across the 4
cores of each modality: replica_groups=[[0,1,2,3],[4,5,6,7]].

Key engine findings from microbenchmarks (mb.py):
  - K=64 matmuls stream moving data at ~2 cycles/col (quarter PE eff);
    zero-padding the contraction to K=128 runs ~1 cycle/col.  Scores
    (contraction rdim=64) therefore use bf16 with rows 64..127 zeroed.
  - fp8e4 DoubleRow matmuls in accumulation chains run ~184 cycles per
    [128,2,128]x[128,2,260] matmul (~3.2x the bf16 per-work rate):
    attn@v runs entirely in fp8 DR (K=256 = two key chunks per matmul).
  - ACT exp is ~1 elem/lane/cycle at 1.2 GHz -> ~109us/core for the 16.8M
    scores; a fraction of exp units runs on DVE instead via the
    "fp8-bits" trick: u8 = round(1.4427*s + B) IS the fp8e4 encoding of
    exp(s/8 - 2) (max rel err ~7%, same order as fp8 quantization).
    Softmax is shift-invariant so the -2 bias cancels; it keeps exp
    outputs under fp8e4's +-240 range.
  - The softmax denominator falls out of the attn@v matmul via a ones
    column appended to v^T (col 256 of each 260-wide padded tile).
"""

import sys
from contextlib import ExitStack

import numpy as np

if "/opt/trn_rl_repo" not in sys.path:
    sys.path.insert(0, "/opt/trn_rl_repo")

import concourse.bass as bass  # noqa: F401
import concourse.mybir as mybir
import concourse.tile as tile
from concourse import bacc
from concourse.bass_utils import run_bass_kernel_spmd
from concourse.masks import make_identity

F32 = mybir.dt.float32
BF16 = mybir.dt.bfloat16
FP8 = mybir.dt.float8e4
U8 = mybir.dt.uint8
DR = mybir.MatmulPerfMode.DoubleRow

DIM = 256          # channels
RDIM = 64          # attention head dim
H = W = 64
N = H * W          # 4096 pixels
B = 4
NCORES = 8
SCALE = RDIM ** -0.5   # 0.125
EPS = 1e-5
NSTAT = B * N      # BN sample count per channel (over all batches)

P = 128            # partitions
CCH = DIM // P     # 2 channel chunks
NQT = 512          # query tile
NT = N // NQT      # 8 query tiles
NKC = N // P       # 32 key chunks of 128
NPAIR = NKC // 2   # 16 key-chunk pairs (= exp units per tile)
NQC = NQT // P     # 4 query sub-chunks per tile
VTW = 260          # 256 channels + ones col (256) + 3 pad (4B aligned)

# exp trick constants: u8 bits = round(A_TRK * s + B_TRK) encode
# fp8e4(exp(s*SCALE + EXPB)); fp8e4 value = 2^(E-7)*(1+M/8), bits = 8E+M.
EXPB = -2.0   # golden scaled scores span [-7.1, 6.7]; e^(6.7-2)=111 < fp8e4 max 240
LOG2E = 1.4426950408889634
A_TRK = 8.0 * SCALE * LOG2E
B_TRK = 8.0 * (EXPB * LOG2E + 7.0) - 0.35   # -0.35 centers the mantissa error

_CACHE = {}
NUM_DEVICES = NCORES
USE_COLLECTIVE = True
NT_RUN = NT
RUN_PROJ = True
REPEAT = 1
LOOP_R = 0
N_DVE_EXP = 6      # exp units per tile computed on DVE (of NPAIR=16)
ACT_V_COPY = True  # (v-proj PSUM->fp8 copies ride ACT; kept for experiments)


def _build_program():
    nc = bacc.Bacc(
        "TRN2",
        target_bir_lowering=False,
        debug=False,
        enable_asserts=False,
        num_devices=NUM_DEVICES,
    )

    xq = nc.dram_tensor("xq", [DIM, N], F32, kind="ExternalInput").ap()
    xkv = nc.dram_tensor("xkv", [DIM, N], F32, kind="ExternalInput").ap()
    wq = nc.dram_tensor("wq", [RDIM, DIM], F32, kind="ExternalInput").ap()
    wk = nc.dram_tensor("wk", [RDIM, DIM], F32, kind="ExternalInput").ap()
    wv = nc.dram_tensor("wv", [DIM, DIM], F32, kind="ExternalInput").ap()
    gamma = nc.dram_tensor("gamma", [DIM], F32, kind="ExternalInput").ap()
    beta = nc.dram_tensor("beta", [DIM], F32, kind="ExternalInput").ap()
    out = nc.dram_tensor("out", [DIM, N], F32, kind="ExternalOutput").ap()

    with tile.TileContext(nc) as tc:
        _body(tc, xq, xkv, wq, wk, wv, gamma, beta, out)

    nc.compile()
    return nc


def _body(tc, xq, xkv, wq, wk, wv, gamma, beta, out):
    nc = tc.nc

    ctx = ExitStack()
    with ctx:
        con = ctx.enter_context(tc.tile_pool(name="con", bufs=1))
        expp = ctx.enter_context(tc.tile_pool(name="expp", bufs=2))
        sml = ctx.enter_context(tc.tile_pool(name="sml", bufs=4))
        dram = ctx.enter_context(tc.tile_pool(name="dram", bufs=1, space="DRAM"))
        ps_s = ctx.enter_context(tc.tile_pool(name="ps_s", bufs=3, space="PSUM"))
        ps_o = ctx.enter_context(tc.tile_pool(name="ps_o", bufs=2, space="PSUM"))

        # ---- persistent SBUF tensors ----
        xq_sb = con.tile([P, CCH * N], F32, name="xq_sb")    # [c%128, cc*N + pix]
        xkv_sb = con.tile([P, CCH * N], F32, name="xkv_sb")
        xq8 = con.tile([P, CCH * N], FP8, name="xq8")        # fp8 shadows
        xkv8 = con.tile([P, CCH * N], FP8, name="xkv8")
        qfb = con.tile([P, N], BF16, name="qfb")             # [rdim|0pad, pix]
        kfb = con.tile([P, N], BF16, name="kfb")             # [rdim|0pad, key]
        vt8 = con.tile([P, NKC * VTW], FP8, name="vt8")      # [key%128, (j, c)]
        wq_raw = con.tile([RDIM, DIM], F32, name="wq_raw")
        wk_raw = con.tile([RDIM, DIM], F32, name="wk_raw")
        wv_raw = con.tile([P, CCH * DIM], F32, name="wv_raw")
        wqT8 = con.tile([P, P], FP8, name="wqT8")            # [cin%128, (kc, rdim)]
        wkT8 = con.tile([P, P], FP8, name="wkT8")
        wvT8 = con.tile([P, CCH * DIM], FP8, name="wvT8")    # [cin%128, (kc, cout)]
        idf = con.tile([P, P], F32, name="idf")
        idb = con.tile([P, P], BF16, name="idb")
        g_sb = con.tile([P, CCH], F32, name="g_sb")
        b_sb = con.tile([P, CCH], F32, name="b_sb")
        sums = con.tile([P, 4], F32, name="sums")
        sums_b = con.tile([P, 4], F32, name="sums_b")
        red_b = con.tile([P, 4], F32, name="red_b")
        part_sum = con.tile([P, CCH * NKC], F32, name="part_sum")
        part_sq = con.tile([P, CCH * NKC], F32, name="part_sq")
        red = con.tile([P, 4], F32, name="red")
        expb_sb = con.tile([P, 1], F32, name="expb_sb")


        stat_in = dram.tile([P, 4], F32, name="stat_in")
        stat_out = dram.tile([P, 4], F32, name="stat_out")
        stat_in_b = dram.tile([P, 4], F32, name="stat_in_b")
        stat_out_b = dram.tile([P, 4], F32, name="stat_out_b")

        # ---- input DMAs (chunked so conversions/projections start early) ----
        nc.sync.dma_start(wq_raw[:], wq[:, :])
        nc.sync.dma_start(wk_raw[:], wk[:, :])
        for cc in range(CCH):
            nc.sync.dma_start(wv_raw[:, cc * DIM:(cc + 1) * DIM], wv[cc * P:(cc + 1) * P, :])
        nc.sync.dma_start(g_sb[:], gamma.rearrange("(k p) -> p k", p=P))
        nc.sync.dma_start(b_sb[:], beta.rearrange("(k p) -> p k", p=P))

        # zero the contraction padding (rows 64..127 of q/k) and v^T extras
        nc.vector.memset(expb_sb[:], EXPB)
        nc.scalar.memzero(qfb[RDIM:P, :])
        nc.scalar.memzero(kfb[RDIM:P, :])
        vt8v = vt8[:].rearrange("p (j c) -> p j c", c=VTW)
        nc.vector.memset(vt8v[:, :, DIM:DIM + 1], 1.0)
        nc.vector.memset(vt8v[:, :, DIM + 1:VTW], 0.0)

        # xkv loads + fp8 shadows (k/v proj need xkv first; xq is loaded
        # between k-proj and q-proj emission, see _emit_attention).
        # shadow conversions split DVE/Pool so they finish fast.
        make_identity(nc, idf[:])
        make_identity(nc, idb[:])

        # ---- transpose + quantize weights on PE ----
        for w_raw, wT8 in ((wq_raw, wqT8), (wk_raw, wkT8)):
            for kc in range(CCH):
                pstf = ps_o.tile([P, 512], F32, tag="pso", name="pstf")
                nc.tensor.transpose(
                    pstf[:, :RDIM], w_raw[:, kc * P:(kc + 1) * P], idf[:RDIM, :RDIM]
                )
                nc.vector.tensor_copy(wT8[:, kc * RDIM:(kc + 1) * RDIM], pstf[:, :RDIM])
        for kc in range(CCH):
            for cc in range(CCH):
                pstf = ps_o.tile([P, 512], F32, tag="pso", name="pstf")
                nc.tensor.transpose(
                    pstf[:, :P], wv_raw[:, cc * DIM + kc * P: cc * DIM + (kc + 1) * P], idf[:]
                )
                nc.vector.tensor_copy(
                    wvT8[:, kc * DIM + cc * P: kc * DIM + (cc + 1) * P], pstf[:, :P]
                )

        split_bn = LOOP_R == 0   # partial-A collective can't live inside For_i
        env_extra = dict(split_bn=split_bn)
        # xkv rides sync/gpsimd queues, xq the scalar HWDGE queue; shadow
        # conversions split DVE/Pool per tile parity.
        for i in range(NT):
            for x_sb, x8, x_dram, q0, q1 in (
                (xkv_sb, xkv8, xkv, nc.sync, nc.gpsimd),
                (xq_sb, xq8, xq, nc.scalar, nc.scalar),
            ):
                for cc in range(CCH):
                    sl = slice(cc * N + i * NQT, cc * N + (i + 1) * NQT)
                    (q0 if cc == 0 else q1).dma_start(
                        x_sb[:, sl], x_dram[cc * P:(cc + 1) * P, i * NQT:(i + 1) * NQT]
                    )
                    if x8 is xq8:
                        nc.scalar.copy(x8[:, sl], x_sb[:, sl])
                    else:
                        ceng = nc.vector if i % 2 == 0 else nc.gpsimd
                        ceng.tensor_copy(x8[:, sl], x_sb[:, sl])

        # ---- compute phases ----
        if LOOP_R > 0:
            with tc.For_i(0, LOOP_R, 1):
                _emit_attention(tc, {**locals(), **env_extra})
            _emit_bn(tc, locals())
        else:
            for _rep in range(REPEAT):
                _emit_attention(tc, {**locals(), **env_extra})
                _emit_bn(tc, locals())


def _emit_attention(tc, env):
    nc = tc.nc
    Exp = mybir.ActivationFunctionType.Exp
    add = mybir.AluOpType.add
    mult = mybir.AluOpType.mult
    xq_sb = env["xq_sb"]; xq8 = env["xq8"]; xkv8 = env["xkv8"]
    qfb = env["qfb"]; kfb = env["kfb"]; vt8 = env["vt8"]
    wqT8 = env["wqT8"]; wkT8 = env["wkT8"]; wvT8 = env["wvT8"]
    idb = env["idb"]
    part_sum = env["part_sum"]; part_sq = env["part_sq"]
    ps_s = env["ps_s"]; ps_o = env["ps_o"]
    sml = env["sml"]; expp = env["expp"]; expb_sb = env["expb_sb"]

    wqT8v = wqT8[:].rearrange("p (kc r) -> p kc r", kc=CCH)
    wkT8v = wkT8[:].rearrange("p (kc r) -> p kc r", kc=CCH)
    wvT8v = wvT8[:].rearrange("p (kc c) -> p kc c", kc=CCH)
    xq8v = xq8[:].rearrange("p (kc x) -> p kc x", kc=CCH)
    xkv8v = xkv8[:].rearrange("p (kc x) -> p kc x", kc=CCH)
    vt8v = vt8[:].rearrange("p (j c) -> p j c", c=VTW)

    # ---- projections (all fp8 DoubleRow, K=256) ----
    for t in range(NT if RUN_PROJ else 0):
        psk = ps_s.tile([RDIM, NQT], F32, tag="ps", name="psk")
        nc.tensor.matmul(
            psk[:], wkT8v[:, :, :], xkv8v[:, :, t * NQT:(t + 1) * NQT],
            start=True, stop=True, perf_mode=DR,
        )
        nc.vector.tensor_copy(kfb[0:RDIM, t * NQT:(t + 1) * NQT], psk[:])
        psq = ps_s.tile([RDIM, NQT], F32, tag="ps", name="psq")
        nc.tensor.matmul(
            psq[:], wqT8v[:, :, :], xq8v[:, :, t * NQT:(t + 1) * NQT],
            start=True, stop=True, perf_mode=DR,
        )
        nc.vector.tensor_copy(qfb[0:RDIM, t * NQT:(t + 1) * NQT], psq[:])

    def emit_vproj_group(g):
        # 4 v chunks into one 2-bank psum slot; one batched fp8 copy on ACT
        psv = ps_s.tile([P, 4 * DIM], F32, tag="ps", name="psv")
        for jj in range(4):
            j = 4 * g + jj
            nc.tensor.matmul(
                psv[:, jj * DIM:(jj + 1) * DIM],
                xkv8v[:, :, j * P:(j + 1) * P], wvT8v[:, :, :],
                start=True, stop=True, perf_mode=DR,
            )
        src = psv[:].rearrange("p (j c) -> p j c", c=DIM)
        nc.scalar.copy(vt8v[:, 4 * g:4 * g + 4, 0:DIM], src)

    if RUN_PROJ and NT_RUN == 0:
        for g in range(NKC // 4):
            emit_vproj_group(g)

    # ---- main loop: software-pipelined over query tiles ----
    # per tile: NSU=11 super-units; super-unit s = scores for key chunks
    # 3s..3s+2 (last one 2 chunks) into half of a manually ring-buffered
    # 6-bank PSUM tensor + ONE exp op over the whole [128,1536] slab (ACT
    # native fp8 out, or the DVE fp8-bits trick).  attn@v chains for tile
    # t-1 interleave between super-units.
    exp_tiles = {}

    def emit_unit(t, u, on_dve):
        exp_t = exp_tiles[t]
        psW = ps_s.tile([P, 2 * NQT], F32, tag="ps", name="psW")
        for h in range(2):
            j = 2 * u + h
            nc.tensor.matmul(
                psW[:, h * NQT:(h + 1) * NQT],
                kfb[:, j * P:(j + 1) * P],
                qfb[:, t * NQT:(t + 1) * NQT],
                start=True, stop=True,
            )
        dst = exp_t[:].rearrange("p (j q) -> p j q", q=NQT)[:, 2 * u:2 * u + 2, :]
        if on_dve:
            nc.vector.tensor_scalar(
                out=dst.bitcast(U8), in0=psW[:],
                scalar1=A_TRK, scalar2=B_TRK, op0=mult, op1=add,
            )
        else:
            nc.scalar.activation(dst, psW[:], Exp, scale=SCALE, bias=expb_sb[:])

    def emit_attnv_mms(t, m, pso, a0, a1):
        expv = exp_tiles[t][:].rearrange("p (j q) -> p j q", q=NQT)
        for a in range(a0, a1):
            nc.tensor.matmul(
                pso[:, 0:VTW], expv[:, 2 * a:2 * a + 2, m * P:(m + 1) * P],
                vt8v[:, 2 * a:2 * a + 2, :],
                start=(a == 0), stop=(a == NPAIR - 1), perf_mode=DR,
                skip_group_check=True,
            )

    def emit_attnv_tail(t, m, pso):
        # tail: normalize, transpose to channel-major, fused residual+stats
        rec = sml.tile([P, 1], F32, tag="rec", name="rec")
        nc.vector.reciprocal(rec[:], pso[:, DIM:DIM + 1])
        ctxn = sml.tile([P, DIM], BF16, tag="ctxn", name="ctxn")
        exp_t = exp_tiles[t]  # noqa: F841  (keeps tile alive until tail)
        nc.vector.tensor_scalar_mul(ctxn[:], pso[:, 0:DIM], rec[:])
        nq0 = t * NQT + m * P
        qi = t * NQC + m
        for cc in range(CCH):
            # transposes live in the tail region of the pso bank (f32 cols
            # 320.. as bf16): temporally disjoint from the attn chain region
            pstt = pso[:, 320 + cc * 64: 384 + cc * 64].bitcast(BF16)
            nc.tensor.transpose(pstt, ctxn[:, cc * P:(cc + 1) * P], idb[:])
            ys = xq_sb[:, cc * N + nq0: cc * N + nq0 + P]
            nc.vector.scalar_tensor_tensor(
                out=ys, in0=pstt, scalar=1.0, in1=ys,
                op0=mult, op1=add,
                accum_out=part_sum[:, cc * NKC + qi: cc * NKC + qi + 1],
            )

    for t in range(NT_RUN + 1):
        cur = t if t < NT_RUN else None
        prev = t - 1 if t >= 1 else None
        if cur is not None:
            exp_tiles[cur] = expp.tile([P, NKC * NQT], FP8, tag="exp", name="exp_t")
        NG = NPAIR // NQC  # units (and attn quarters) per m-group
        for m in range(NQC):
            pso = (ps_o.tile([P, 512], F32, tag="pso", name="pso")
                   if prev is not None else None)
            for g in range(NG):
                if cur is not None:
                    u = m * NG + g
                    # spread N_DVE_EXP DVE units evenly across the tile
                    on_dve = (u * N_DVE_EXP) // NPAIR != ((u + 1) * N_DVE_EXP) // NPAIR
                    emit_unit(cur, u, on_dve)
                    if RUN_PROJ and cur == 0 and u % 2 == 1:
                        emit_vproj_group(u // 2)
                if prev is not None:
                    emit_attnv_mms(prev, m, pso, g * NG, (g + 1) * NG)
            if prev is not None:
                emit_attnv_tail(prev, m, pso)
        if prev is not None:
            sq_scr = sml.tile([P, NQT], F32, tag="sq_scr", name="sq_scr")
            for cc in range(CCH):
                ysl = xq_sb[:, cc * N + prev * NQT: cc * N + (prev + 1) * NQT]
                nc.vector.scalar_tensor_tensor(
                    out=sq_scr[:], in0=ysl, scalar=1.0, in1=ysl,
                    op0=mult, op1=mult,
                    accum_out=part_sq[:, cc * NT + prev: cc * NT + prev + 1],
                )
            exp_tiles.pop(prev)
        if env.get("split_bn") and t == NT_RUN - 1 and NT_RUN == NT:
            # stats for tiles 0..NT-2 are final once tile NT-2's tails ran;
            # fire the big AllReduce now so it overlaps the last tile.
            sums = env["sums"]; stat_in = env["stat_in"]; stat_out = env["stat_out"]
            AX = mybir.AxisListType.X
            for cc in range(CCH):
                nc.vector.reduce_sum(
                    sums[:, cc:cc + 1],
                    part_sum[:, cc * NKC: cc * NKC + (NT - 1) * NQC], axis=AX,
                )
                nc.vector.reduce_sum(
                    sums[:, 2 + cc:3 + cc],
                    part_sq[:, cc * NT: cc * NT + NT - 1], axis=AX,
                )
            nc.gpsimd.dma_start(stat_in[:], sums[:])
            if USE_COLLECTIVE:
                nc.gpsimd.collective_compute(
                    "AllReduce", add,
                    ins=[stat_in[:]], outs=[stat_out[:]],
                    replica_groups=[[0, 1, 2, 3], [4, 5, 6, 7]],
                )
            else:
                nc.gpsimd.dma_start(stat_out[:], stat_in[:])
            nc.gpsimd.dma_start(env["red"][:], stat_out[:])


def _emit_bn(tc, env):
    nc = tc.nc
    Sqrt = mybir.ActivationFunctionType.Sqrt
    add = mybir.AluOpType.add
    mult = mybir.AluOpType.mult
    subtract = mybir.AluOpType.subtract
    AX = mybir.AxisListType.X
    xq_sb = env["xq_sb"]
    g_sb = env["g_sb"]; b_sb = env["b_sb"]
    sums = env["sums"]; red = env["red"]; stat_in = env["stat_in"]; stat_out = env["stat_out"]
    sml = env["sml"]
    part_sum = env["part_sum"]; part_sq = env["part_sq"]

    split_bn = env.get("split_bn")
    if split_bn:
        sums_b = env["sums_b"]; red_b = env["red_b"]
        stat_in_b = env["stat_in_b"]; stat_out_b = env["stat_out_b"]
        for cc in range(CCH):
            nc.vector.reduce_sum(
                sums_b[:, cc:cc + 1],
                part_sum[:, cc * NKC + (NT - 1) * NQC: cc * NKC + NT * NQC],
                axis=AX,
            )
            nc.vector.tensor_copy(
                sums_b[:, 2 + cc:3 + cc],
                part_sq[:, cc * NT + NT - 1: cc * NT + NT],
            )
        nc.gpsimd.dma_start(stat_in_b[:], sums_b[:])
        if USE_COLLECTIVE:
            nc.gpsimd.collective_compute(
                "AllReduce", add,
                ins=[stat_in_b[:]], outs=[stat_out_b[:]],
                replica_groups=[[0, 1, 2, 3], [4, 5, 6, 7]],
            )
        else:
            nc.gpsimd.dma_start(stat_out_b[:], stat_in_b[:])
        nc.gpsimd.dma_start(red_b[:], stat_out_b[:])
        nc.vector.tensor_tensor(out=red[:], in0=red[:], in1=red_b[:], op=add)
    else:
        for cc in range(CCH):
            nc.vector.reduce_sum(
                sums[:, cc:cc + 1], part_sum[:, cc * NKC:(cc + 1) * NKC], axis=AX
            )
            nc.vector.reduce_sum(
                sums[:, 2 + cc:3 + cc], part_sq[:, cc * NT:(cc + 1) * NT], axis=AX
            )
        nc.gpsimd.dma_start(stat_in[:], sums[:])
        if USE_COLLECTIVE:
            nc.gpsimd.collective_compute(
                "AllReduce", add,
                ins=[stat_in[:]], outs=[stat_out[:]],
                replica_groups=[[0, 1, 2, 3], [4, 5, 6, 7]],
            )
        else:
            nc.gpsimd.dma_start(stat_out[:], stat_in[:])
        nc.gpsimd.dma_start(red[:], stat_out[:])

    meanv = sml.tile([P, CCH], F32, tag="meanv", name="meanv")
    es2 = sml.tile([P, CCH], F32, tag="es2", name="es2")
    varp = sml.tile([P, CCH], F32, tag="varp", name="varp")
    rstd = sml.tile([P, CCH], F32, tag="rstd", name="rstd")
    scl = sml.tile([P, CCH], F32, tag="scl", name="scl")
    sh = sml.tile([P, CCH], F32, tag="sh", name="sh")
    nc.vector.tensor_scalar_mul(meanv[:], red[:, 0:CCH], 1.0 / NSTAT)
    nc.vector.tensor_scalar_mul(es2[:], red[:, CCH:2 * CCH], 1.0 / NSTAT)
    nc.vector.tensor_tensor(out=varp[:], in0=meanv[:], in1=meanv[:], op=mult)
    nc.vector.tensor_tensor(out=varp[:], in0=es2[:], in1=varp[:], op=subtract)
    nc.vector.tensor_scalar_add(varp[:], varp[:], EPS)
    nc.scalar.activation(rstd[:], varp[:], Sqrt)
    nc.vector.reciprocal(rstd[:], rstd[:])
    nc.vector.tensor_tensor(out=scl[:], in0=g_sb[:], in1=rstd[:], op=mult)
    nc.vector.tensor_tensor(out=sh[:], in0=meanv[:], in1=scl[:], op=mult)
    nc.vector.tensor_tensor(out=sh[:], in0=b_sb[:], in1=sh[:], op=subtract)

    out = env["out"]
    Identity = mybir.ActivationFunctionType.Identity
    for i in range(NT):
        for cc in range(CCH):
            s = slice(cc * N + i * NQT, cc * N + (i + 1) * NQT)
            if cc == 0:
                nc.scalar.activation(
                    xq_sb[:, s], xq_sb[:, s], Identity,
                    scale=scl[:, cc:cc + 1], bias=sh[:, cc:cc + 1],
                )
            else:
                nc.vector.tensor_scalar(
                    out=xq_sb[:, s], in0=xq_sb[:, s],
                    scalar1=scl[:, cc:cc + 1], scalar2=sh[:, cc:cc + 1],
                    op0=mult, op1=add,
                )
            (nc.sync, nc.gpsimd, nc.scalar)[(i * CCH + cc) % 3].dma_start(
                out[cc * P:(cc + 1) * P, i * NQT:(i + 1) * NQT], xq_sb[:, s]
            )


def get_program():
    if "nc" not in _CACHE:
        _CACHE["nc"] = _build_program()
    return _CACHE["nc"]


def make_in_maps(wli_feat, nbi_aligned, w_q_wli, w_k_nbi, w_v_nbi,
                 w_q_nbi, w_k_wli, w_v_wli, gamma_wli, beta_wli,
                 gamma_nbi, beta_nbi):
    """Per-core inputs: cores 0..3 = (batch b, wli), cores 4..7 = (batch b, nbi),
    so modality groups are {0,1,2,3} (wli) and {4,5,6,7} (nbi)."""

    def f(x):
        return np.ascontiguousarray(np.asarray(x, dtype=np.float32))

    in_maps = []
    for b in range(B):
        in_maps.append({
            "xq": f(wli_feat[b]).reshape(DIM, N),
            "xkv": f(nbi_aligned[b]).reshape(DIM, N),
            "wq": f(w_q_wli), "wk": f(w_k_nbi), "wv": f(w_v_nbi),
            "gamma": f(gamma_wli), "beta": f(beta_wli),
        })
    for b in range(B):
        in_maps.append({
            "xq": f(nbi_aligned[b]).reshape(DIM, N),
            "xkv": f(wli_feat[b]).reshape(DIM, N),
            "wq": f(w_q_nbi), "wk": f(w_k_wli), "wv": f(w_v_wli),
            "gamma": f(gamma_nbi), "beta": f(beta_nbi),
        })
    return in_maps


def assemble_outputs(results):
    enh_wli = np.empty((B, DIM, H, W), np.float32)
    enh_nbi = np.empty((B, DIM, H, W), np.float32)
    for b in range(B):
        enh_wli[b] = results[b]["out"].reshape(DIM, H, W)
        enh_nbi[b] = results[B + b]["out"].reshape(DIM, H, W)
    return enh_wli, enh_nbi


def kernel(**inputs):
    nc = get_program()
    in_maps = make_in_maps(**inputs)
    res = run_bass_kernel_spmd(nc, in_maps, list(range(NCORES)))
    return assemble_outputs(res.results)
